# revision 1
# baseline (speedup 1.0000x reference)
"""DeformAlign Trainium2 kernel (one image per NeuronCore, 8-way batch parallel).

Pipeline per core:
  conv1 (3x3, relu, PE fp16) -> conv2 (3x3, PE) -> offsets
  offsets -> idx-major fields (floor/frac/validity/bilinear weights, DVE)
  x -> fp16 token table in DRAM (records = vertical channel strips, x-pairs)
  dma_gather: 1 descriptor per (tap,pixel) fetches all 4 bilinear corners
  DVE combine (4 broadcast-mults + adds) -> sampled
  PE transpose -> einsum out[o,p] = sum_{c,k} W[o,c,k] sampled[c,k,p]
"""
import contextlib
import numpy as np

import concourse.bass as bass
import concourse.bacc as bacc
import concourse.mybir as mybir
from concourse.tile import TileContext
from concourse import library_config

F32 = mybir.dt.float32
F16 = mybir.dt.float16
I16 = mybir.dt.int16
I32 = mybir.dt.int32
AL = mybir.AluOpType
AF = mybir.ActivationFunctionType

H = W = 128
C = 64
O = 64
HW = H * W
PAD = 130
PHW = PAD * PAD
RPC = 131            # table records per x-column
NRECT = 16900        # table records
RECW = 128           # fp16 elements per record
ELEM = 256           # gather elem: 2 vertically-adjacent records
NCHUNK = 16
CH_Y = 8
CH_PIX = CH_Y * W    # 1024
NIDXC = 9 * CH_PIX   # 9216 indices per chunk
CCH = 512            # conv sweep chunk
P0_FIRST = PAD + 1               # first interior padded position
P_END = PAD * (H + 1) - 1        # one past last interior padded position


def bcast_ap(sliced, n):
    """Append a [0, n] broadcast dim to a sliced AP."""
    return bass.AP(sliced.tensor, sliced.offset, list(sliced.ap) + [[0, n]])


def build_nc(phase=99):
    nc = bacc.Bacc("TRN2", target_bir_lowering=False)

    x_in = nc.dram_tensor("x", [C, HW], F32, kind="ExternalInput")
    ref_in = nc.dram_tensor("ref", [C, HW], F32, kind="ExternalInput")
    w1_in = nc.dram_tensor("w1", [64 * C * 9], F32, kind="ExternalInput")
    b1_in = nc.dram_tensor("b1", [64], F32, kind="ExternalInput")
    w2_in = nc.dram_tensor("w2", [18 * C * 9], F32, kind="ExternalInput")
    b2_in = nc.dram_tensor("b2", [18], F32, kind="ExternalInput")
    wt_in = nc.dram_tensor("wt", [O * C * 9], F32, kind="ExternalInput")
    ident_in = nc.dram_tensor("ident", [128, 128], F16, kind="ExternalInput")
    yb_in = nc.dram_tensor("ybias", [128, 3, 128], F32, kind="ExternalInput")
    xb_in = nc.dram_tensor("xbias", [128, 3, 128], F32, kind="ExternalInput")

    out_t = nc.dram_tensor("out", [O, HW], F32, kind="ExternalOutput")

    with TileContext(nc) as tc, contextlib.ExitStack() as ctx:
        pool = ctx.enter_context(tc.tile_pool(name="sb", bufs=1))
        bigp = ctx.enter_context(tc.tile_pool(name="bg", bufs=3))
        rpool = ctx.enter_context(tc.tile_pool(name="rb", bufs=2))
        ppool = ctx.enter_context(tc.tile_pool(name="ps", bufs=2, space="PSUM"))
        peins = ctx.enter_context(tc.tile_pool(name="pe", bufs=2, space="PSUM"))
        dpool = ctx.enter_context(tc.tile_pool(name="dr", bufs=1, space="DRAM"))


        # ---------------- constants ----------------
        ident = pool.tile([128, 128], F16)
        nc.sync.dma_start(ident[:], ident_in[:])
        ybias = pool.tile([128, 3, 128], F32)
        nc.sync.dma_start(ybias[:], yb_in[:])
        xbias = pool.tile([128, 3, 128], F32)
        nc.sync.dma_start(xbias[:], xb_in[:])
        b1t = pool.tile([64, 1], F32)
        nc.sync.dma_start(b1t[:], bass.AP(b1_in, 0, [[1, 64], [1, 1]]))
        b2t = pool.tile([18, 1], F32)
        nc.sync.dma_start(b2t[:], bass.AP(b2_in, 0, [[1, 18], [1, 1]]))

        # conv1 weights: pairs (kj=0,1) and singles (kj=2) per ki
        # flat w1 offset = o*576 + c*9 + ki*3 + kj
        w1l = pool.tile([128, 3, 64], F16)
        w1s = pool.tile([64, 3, 64], F16)
        w2l = pool.tile([128, 3, 18], F16)
        w2s = pool.tile([64, 3, 18], F16)
        for ki in range(3):
            for krel in range(2):
                nc.gpsimd.dma_start(w1l[64 * krel:64 * (krel + 1), ki, :],
                                    bass.AP(w1_in, ki * 3 + krel, [[9, 64], [576, 64]]))
                nc.gpsimd.dma_start(w2l[64 * krel:64 * (krel + 1), ki, :],
                                    bass.AP(w2_in, ki * 3 + krel, [[9, 64], [576, 18]]))
            nc.gpsimd.dma_start(w1s[:, ki, :],
                                bass.AP(w1_in, ki * 3 + 2, [[9, 64], [576, 64]]))
            nc.gpsimd.dma_start(w2s[:, ki, :],
                                bass.AP(w2_in, ki * 3 + 2, [[9, 64], [576, 18]]))

        # einsum weights: tap pairs (0,1)(2,3)(4,5)(6,7) + single 8
        wel = pool.tile([128, 4, 64], F16)
        wes = pool.tile([64, 1, 64], F16)
        for pr in range(4):
            for krel in range(2):
                nc.gpsimd.dma_start(wel[64 * krel:64 * (krel + 1), pr, :],
                                    bass.AP(wt_in, pr * 2 + krel, [[9, 64], [576, 64]]))
        nc.gpsimd.dma_start(wes[:, 0, :], bass.AP(wt_in, 8, [[9, 64], [576, 64]]))

        # ---------------- x -> fp16, transpose, token table ----------------
        xh = bigp.tile([64, HW], F16, tag="big")
        nc.gpsimd.dma_start(xh[:], x_in[:])

        # xTh[xcol, y, c] = x[c, y, xcol]
        xTh = bigp.tile([128, H, C], F16, tag="big")
        for blk in range(16):
            tps = ppool.tile([128, 512], F16, tag="tph")
            for j in range(8):
                y = blk * 8 + j
                nc.tensor.transpose(tps[:, j * C:(j + 1) * C],
                                    xh[:, y * W:(y + 1) * W],
                                    ident[0:64, 0:64])
            nc.scalar.activation(xTh[:, blk * 8:(blk + 1) * 8, :], tps[:], AF.Copy)

        # token table: record (cx, yrow) at q = cx*131 + yrow, cx = xcol+1
        #   slot0 (els 0:64)  = x[c, yrow-1, cx-1]
        #   slot1 (els 64:128)= x[c, yrow-1, cx]
        table = dpool.tile([NRECT, RECW], F16)
        ztile = pool.tile([128, 192], F16)
        nc.vector.memset(ztile[:], 0.0)
        # border zeros: cx=0 column slot0 (131 recs); cx=128 column slot1;
        # yrow=0 / 129 / 130 rows (129 columns each)
        nc.sync.dma_start(bass.AP(table.tensor, table.offset, [[RECW, 128], [1, 64]]),
                          ztile[:, 0:64])
        nc.sync.dma_start(bass.AP(table.tensor, table.offset + 128 * RECW, [[RECW, 3], [1, 64]]),
                          ztile[0:3, 0:64])
        nc.sync.dma_start(bass.AP(table.tensor, table.offset + 128 * RPC * RECW + 64,
                                  [[RECW, 128], [1, 64]]),
                          ztile[:, 0:64])
        nc.sync.dma_start(bass.AP(table.tensor, table.offset + (128 * RPC + 128) * RECW + 64,
                                  [[RECW, 3], [1, 64]]),
                          ztile[0:3, 0:64])
        for yr in (0, 129, 130):
            nc.sync.dma_start(bass.AP(table.tensor, table.offset + yr * RECW,
                                      [[RPC * RECW, 128], [1, RECW]]),
                              ztile[:, 0:128])
            nc.sync.dma_start(bass.AP(table.tensor, table.offset + (128 * RPC + yr) * RECW,
                                      [[1, 1], [1, RECW]]),
                              ztile[0:1, 0:128])
        # main fills (yrow = y+1 for y in 0..127)
        nc.sync.dma_start(bass.AP(table.tensor, table.offset + (RPC + 1) * RECW,
                                  [[RPC * RECW, 128], [RECW, H], [1, C]]),
                          xTh[:])  # slot0: cx = xcol+1 (partitions = xcol)
        nc.sync.dma_start(bass.AP(table.tensor, table.offset + RECW + 64,
                                  [[RPC * RECW, 128], [RECW, H], [1, C]]),
                          xTh[:])  # slot1: cx = xcol   (partitions = xcol)

        # ---------------- conv1 ----------------
        refdup = bigp.tile([128, PHW + 4], F16, tag="big")
        nc.vector.memset(refdup[:], 0.0)
        nc.gpsimd.dma_start(
            refdup[0:64, 0:PHW].rearrange("c (y x) -> c y x", y=PAD)[:, 1:129, 1:129],
            ref_in[:].rearrange("c (h w) -> c h w", h=H))
        nc.gpsimd.dma_start(
            refdup[64:128, 0:PHW].rearrange("c (y x) -> c y x", y=PAD)[:, 1:129, 0:128],
            ref_in[:].rearrange("c (h w) -> c h w", h=H))

        t1p = bigp.tile([64, PHW], F16, tag="big")
        nc.vector.memset(t1p[:], 0.0)
        for j in range((P_END - P0_FIRST + CCH - 1) // CCH):
            p0 = P0_FIRST + j * CCH
            n = min(CCH, P_END - p0)
            ps = ppool.tile([128, CCH], F32, tag="tp")
            for ki in range(3):
                d0 = (ki - 1) * PAD - 1
                nc.tensor.matmul(ps[0:64, 0:n], w1l[:, ki, :],
                                 refdup[:, p0 + d0: p0 + d0 + n],
                                 start=(ki == 0), stop=False)
                nc.tensor.matmul(ps[0:64, 0:n], w1s[:, ki, :],
                                 refdup[0:64, p0 + d0 + 2: p0 + d0 + 2 + n],
                                 start=False, stop=(ki == 2))
            nc.scalar.activation(t1p[:, p0:p0 + n], ps[0:64, 0:n], AF.Relu, bias=b1t[:])
        # re-zero border columns of t1p
        t1v = t1p[:].rearrange("c (y x) -> c y x", y=PAD)
        nc.vector.memset(t1v[:, :, 0:1], 0.0)
        nc.vector.memset(t1v[:, :, 129:130], 0.0)

        # t1 duplicated (+1 shifted) for conv2 tap pairing
        t1d = bigp.tile([128, PHW + 4], F16, tag="big")
        nc.vector.memset(t1d[:], 0.0)
        nc.sync.dma_start(t1d[0:64, 0:PHW], t1p[:])
        nc.sync.dma_start(t1d[64:128, 0:PHW - 1], t1p[:, 1:PHW])

        # ---------------- conv2 -> offp (fp16, padded layout) ----------------
        offp = bigp.tile([18, PHW], F16, tag="big")
        for j in range((P_END - P0_FIRST + CCH - 1) // CCH):
            p0 = P0_FIRST + j * CCH
            n = min(CCH, P_END - p0)
            ps = ppool.tile([128, CCH], F32, tag="tp")
            for ki in range(3):
                d0 = (ki - 1) * PAD - 1
                nc.tensor.matmul(ps[0:18, 0:n], w2l[:, ki, :],
                                 t1d[:, p0 + d0: p0 + d0 + n],
                                 start=(ki == 0), stop=False)
                nc.tensor.matmul(ps[0:18, 0:n], w2s[:, ki, :],
                                 t1d[0:64, p0 + d0 + 2: p0 + d0 + 2 + n],
                                 start=False, stop=(ki == 2))
            nc.scalar.activation(offp[:, p0:p0 + n], ps[0:18, 0:n], AF.Identity, bias=b2t[:])

        # ---------------- offsets -> idx-major offT[x, y, ch] (f32) ----------------
        offT = pool.tile([128, H, 18], F16)
        for blk in range(16):
            tps = ppool.tile([128, 512], F16, tag="tph")
            for j in range(8):
                y = blk * 8 + j
                nc.tensor.transpose(tps[:, j * 18:(j + 1) * 18],
                                    offp[:, (y + 1) * PAD + 1:(y + 1) * PAD + 1 + W],
                                    ident[0:18, 0:18])
            nc.scalar.activation(offT[:, blk * 8:(blk + 1) * 8, :],
                                 tps[:, 0:8 * 18], AF.Copy)

        if phase == 0:
            dbg = pool.tile([64, 4608], F32)
            nc.scalar.activation(dbg[:], offT[0:64, :, :], AF.Copy)
            nc.sync.dma_start(out_t[:, 0:4608], dbg[:])
            nc.vector.memset(dbg[:], 0.0)
            for j in range(3):
                nc.sync.dma_start(out_t[:, 4608 * (j + 1):4608 * (j + 2)], dbg[:])
            nc.compile()
            return nc

        # ---------------- per-tap fields ----------------
        wf = pool.tile([128, 4, 9, H], F16)   # corner weights (00,01,10,11)
        qi = pool.tile([128, NCHUNK, 9, CH_Y], I16)  # token indices, y-chunked
        tmpa = pool.tile([128, H], F32)
        tmpb = pool.tile([128, H], F32)
        yw0 = pool.tile([128, H], F32)
        yw1 = pool.tile([128, H], F32)
        xw0 = pool.tile([128, H], F32)
        xw1 = pool.tile([128, H], F32)
        vm = pool.tile([128, H], F32)
        qf = pool.tile([128, H], F32)

        for k in range(9):
            ki, kj = k // 3, k % 3
            for axis in range(2):
                ob = offT[:, :, 2 * k + axis]
                bias = ybias[:, ki, :] if axis == 0 else xbias[:, kj, :]
                w0, w1_ = (yw0, yw1) if axis == 0 else (xw0, xw1)
                nc.vector.tensor_tensor(tmpa[:], ob, bias, op=AL.add)     # biased coord
                # floor via RNE-to-integer (magic add) + round-up correction
                nc.vector.tensor_scalar(tmpb[:], tmpa[:], 12582912.0, -12582912.0,
                                        op0=AL.add, op1=AL.add)
                nc.vector.tensor_tensor(vm[:], tmpb[:], tmpa[:], op=AL.is_gt)
                nc.vector.tensor_tensor(tmpb[:], tmpb[:], vm[:], op=AL.subtract)
                nc.vector.tensor_tensor(w1_[:], tmpa[:], tmpb[:], op=AL.subtract)  # frac
                # corner0 validity: floor in [8, 135]
                nc.vector.tensor_scalar(vm[:], tmpb[:], 8.0, None, op0=AL.is_ge)
                nc.vector.tensor_scalar(tmpa[:], tmpb[:], 135.0, None, op0=AL.is_le)
                nc.vector.tensor_tensor(vm[:], vm[:], tmpa[:], op=AL.mult)
                nc.vector.tensor_scalar(w0[:], w1_[:], -1.0, 1.0, op0=AL.mult, op1=AL.add)
                nc.vector.tensor_tensor(w0[:], w0[:], vm[:], op=AL.mult)
                # corner1 validity: floor in [7, 134]
                nc.vector.tensor_scalar(vm[:], tmpb[:], 7.0, None, op0=AL.is_ge)
                nc.vector.tensor_scalar(tmpa[:], tmpb[:], 134.0, None, op0=AL.is_le)
                nc.vector.tensor_tensor(vm[:], vm[:], tmpa[:], op=AL.mult)
                nc.vector.tensor_tensor(w1_[:], w1_[:], vm[:], op=AL.mult)
                # clamped floor -> q
                nc.vector.tensor_scalar(tmpa[:], tmpb[:], 7.0, 135.0, op0=AL.max, op1=AL.min)
                if axis == 0:
                    nc.vector.tensor_copy(qf[:], tmpa[:])
                else:
                    nc.vector.tensor_scalar(tmpa[:], tmpa[:], 131.0, -924.0,
                                            op0=AL.mult, op1=AL.add)
                    nc.vector.tensor_tensor(qf[:], qf[:], tmpa[:], op=AL.add)
            nc.vector.tensor_copy(qi[:, :, k, :], qf[:].rearrange("p (c y) -> p c y", c=NCHUNK))
            nc.vector.tensor_tensor(wf[:, 0, k, :], yw0[:], xw0[:], op=AL.mult)
            nc.vector.tensor_tensor(wf[:, 1, k, :], yw0[:], xw1[:], op=AL.mult)
            nc.vector.tensor_tensor(wf[:, 2, k, :], yw1[:], xw0[:], op=AL.mult)
            nc.vector.tensor_tensor(wf[:, 3, k, :], yw1[:], xw1[:], op=AL.mult)

        # ---------------- wrapped idx ----------------
        wrp = pool.tile([128, NCHUNK * 9 * CH_Y * 8], I16)
        wrv = wrp[0:16, :].rearrange("p (c k y f) -> p c k y f",
                                     c=NCHUNK, k=9, y=CH_Y, f=8)
        for ph in range(8):
            nc.sync.dma_start(wrv[:, :, :, :, ph],
                              qi[16 * ph:16 * (ph + 1), :, :, :])
        nc.sync.dma_start(wrp[16:32, :], wrp[0:16, :])
        nc.sync.dma_start(wrp[32:64, :], wrp[0:32, :])
        nc.sync.dma_start(wrp[64:128, :], wrp[0:64, :])

        if phase == 1:
            dbg2 = pool.tile([64, 9216], F32)
            nc.gpsimd.dma_start(dbg2[:], wf[:].rearrange("p a b c -> p (a b c)"))
            nc.sync.dma_start(out_t[:, 0:9216], dbg2[0:64, 0:9216])
            nc.vector.memset(dbg2[:], 0.0)
            nc.sync.dma_start(out_t[:, 9216:16384], dbg2[0:64, 0:16384 - 9216])
            nc.compile()
            return nc

        # ---------------- gather / combine / transpose / einsum ----------------
        table_ap = bass.AP(table.tensor, table.offset, [[RECW, NRECT - 2], [1, ELEM]])
        NW = NCHUNK * 9 * CH_Y * 8 // NCHUNK  # idx cols per chunk = 576

        for cnk in range(1 if phase == 2 else NCHUNK):
            g = bigp.tile([128, NIDXC // 128, ELEM], F16, tag="big")
            GSZ = 512
            for sub in range(NIDXC // GSZ):
                nc.gpsimd.dma_gather(
                    g[:, sub * (GSZ // 128):(sub + 1) * (GSZ // 128), :], table_ap,
                    wrp[:, cnk * NW + sub * (GSZ // 16):
                        cnk * NW + (sub + 1) * (GSZ // 16)],
                    GSZ, GSZ, ELEM, elem_step=RECW)

            s = rpool.tile([128, 9 * CH_Y, C], F16, tag="s")
            m = rpool.tile([128, CH_Y, C], F16, tag="m", bufs=1)
            for k in range(9):
                sk = s[:, k * CH_Y:(k + 1) * CH_Y, :]
                for corner in range(4):
                    rowj, slot = corner // 2, corner % 2
                    off0 = rowj * 128 + slot * 64
                    gco = g[:, k * CH_Y:(k + 1) * CH_Y, off0:off0 + 64]
                    wco = bcast_ap(wf[:, corner, k, cnk * CH_Y:(cnk + 1) * CH_Y], C)
                    if corner == 0:
                        nc.vector.tensor_tensor(sk, gco, wco, op=AL.mult)
                    else:
                        nc.vector.tensor_tensor(m[:], gco, wco, op=AL.mult)
                        nc.vector.tensor_tensor(sk, sk, m[:], op=AL.add)

            rhs_p = [rpool.tile([128, CH_PIX], F16, tag=f"rp{i}", name=f"rhs_p{i}", bufs=1) for i in range(4)]
            rhs_s = rpool.tile([64, CH_PIX], F16, tag="rs", bufs=1)
            for k in range(9):
                tps = ppool.tile([128, 512], F16, tag="tph")
                for pr in range(4):
                    nc.tensor.transpose(
                        tps[:, pr * 128:(pr + 1) * 128],
                        s[:, k * CH_Y + pr * 2:k * CH_Y + pr * 2 + 2, :].rearrange(
                            "p a c -> p (a c)"),
                        ident[:, :])
                dst, prow = (rhs_s, 0) if k == 8 else (rhs_p[k // 2], 64 * (k % 2))
                dv = dst[prow:prow + 64, :].rearrange("c (pp j x) -> c pp j x",
                                                      pp=4, j=2, x=W)
                for jr in range(2):
                    nc.scalar.activation(dv[:, :, jr, :],
                                         tps[jr * 64:(jr + 1) * 64, :], AF.Copy)

            pso = peins.tile([64, CH_PIX], F32, tag="eo")
            for half in range(2):
                colr = slice(half * 512, (half + 1) * 512)
                for pr in range(4):
                    nc.tensor.matmul(pso[:, colr], wel[:, pr, :], rhs_p[pr][:, colr],
                                     start=(pr == 0), stop=False)
                nc.tensor.matmul(pso[:, colr], wes[:, 0, :], rhs_s[:, colr],
                                 start=False, stop=True)
            ost = rpool.tile([64, CH_PIX], F16, tag="os")
            nc.scalar.activation(ost[:], pso[:], AF.Copy)
            nc.gpsimd.dma_start(out_t[:, cnk * CH_PIX:(cnk + 1) * CH_PIX], ost[:])

    nc.compile()
    return nc


# ----------------------------------------------------------------------------
# host-side wrapper
# ----------------------------------------------------------------------------
def host_consts():
    ident = np.eye(128, dtype=np.float16)
    yb = np.zeros((128, 3, 128), np.float32)
    xb = np.zeros((128, 3, 128), np.float32)
    for j in range(3):
        yb[:, j, :] = np.arange(128, dtype=np.float32)[None, :] + (j - 1) + 8
        xb[:, j, :] = np.arange(128, dtype=np.float32)[:, None] + (j - 1) + 8
    return ident, yb, xb


def make_in_map(xb_img, ref_img, w1, b1, w2, b2, weight):
    ident, yb, xbias = host_consts()
    return {
        "x": np.ascontiguousarray(xb_img.reshape(C, HW), np.float32),
        "ref": np.ascontiguousarray(ref_img.reshape(C, HW), np.float32),
        "w1": np.ascontiguousarray(w1.reshape(-1), np.float32),
        "b1": np.ascontiguousarray(b1, np.float32),
        "w2": np.ascontiguousarray(w2.reshape(-1), np.float32),
        "b2": np.ascontiguousarray(b2, np.float32),
        "wt": np.ascontiguousarray(weight.reshape(-1), np.float32),
        "ident": ident,
        "ybias": yb,
        "xbias": xbias,
    }


_NC_CACHE = None
TRACE = False
LAST_EXEC_NS = None


def kernel(x, ref_feature, w1, b1, w2, b2, weight):
    """Full-input entry point: shard batch across 8 cores, gather output."""
    global _NC_CACHE, LAST_EXEC_NS
    from concourse.bass_utils import run_bass_kernel_spmd
    x = np.asarray(x)
    ref_feature = np.asarray(ref_feature)
    B = x.shape[0]
    assert B == 8
    if _NC_CACHE is None:
        _NC_CACHE = build_nc()
    nc = _NC_CACHE
    in_maps = [make_in_map(x[b], ref_feature[b], w1, b1, w2, b2, weight)
               for b in range(B)]
    res = run_bass_kernel_spmd(nc, in_maps, core_ids=list(range(8)), trace=TRACE)
    LAST_EXEC_NS = res.exec_time_ns
    out = np.stack([res.results[b]["out"].reshape(O, H, W) for b in range(B)])
    return out.astype(np.float32)

